# revision 11
# baseline (speedup 1.0000x reference)
"""Trainium2 Bass kernel for hash-indexed im2col gather + GEMM.

Reference computation:
    gathered = x.reshape(B, -1)[:, hashtable]        # [B, HW, K*C]
    out = einsum('nq,bpq->bnp', weights, gathered)   # [B, KN, HW]
    return out.reshape(B, KN, H, W)

Strategy (pixel-parallel over 8 NeuronCores; all 16 batch images processed
together since the hashtable is shared across the batch):
  - Host: transpose x to xt [1048576, 16] f32 so each flat index is one
    contiguous 64B row holding all 16 batch values.
  - Each NC owns 2048 pixels, processed in 128-pixel tiles. For each tile
    and each SBUF partition rho, one SWDGE indirect DMA gathers that
    partition's contraction rows (q = c*128 + rho, c = 0..3) for all tile
    pixels, landing directly in matmul-ready layout G[rho, (c, sub, px, b)].
    The 5th contraction chunk (q = 512+rho4, rho4 < 64) is gathered by
    separate instructions packing 4 tiles each.
  - The SWDGE indirect path corrupts the first descriptor of each DMA
    engine's run (it reads the index L/16*e... positions displaced), i.e.
    every L/16-th stream slot. All instructions are exactly 640 slots so
    the corruption period is 40, and streams are built from 40-slot blocks
    of [8 pad + 32 real]: corrupted slots always land on pads.
  - TensorE contracts q in 5 chunks (4x128 + 64) accumulating in PSUM
    [64, 32px*16b]; ScalarE copies PSUM->SBUF; HWDGE writes DRAM.
"""
import sys
import numpy as np

if "/opt/trn_rl_repo" not in sys.path:
    sys.path.insert(0, "/opt/trn_rl_repo")

import concourse.bacc as bacc
import concourse.bass as bass
import concourse.mybir as mybir
from concourse._compat import get_trn_type
from concourse.bass_utils import run_bass_kernel_spmd

B, C, H, W = 16, 64, 128, 128
HW = H * W
Q = 576
KN = 64
NCORES = 8
BLK = 40          # stream block: 8 pad + 32 real rows
PAD = 8
GRP = 32          # pixels per matmul sub-group
LSLOT = 640       # slots per gather instruction (16 blocks)
WCOLS = LSLOT // 128   # 5 idx columns per instruction


class Cfg:
    def __init__(self, px_nc=2048, rows=C * H * W):
        self.px_nc = px_nc
        self.px_tile = 128
        self.ntiles = px_nc // self.px_tile           # 16
        self.nsuper = self.ntiles // 4                # 4
        self.nsub = self.px_tile // GRP               # 4
        self.idx_cols = 128 * WCOLS                   # 640 (main, per tile)
        self.idx4_cols = 64 * WCOLS                   # 320 (chunk4, per super)
        self.gw = LSLOT * B                           # 10240 f32 per partition
        self.rows = rows


_cached = {}


def build(cfg: Cfg):
    nc = bacc.Bacc(get_trn_type() or "TRN2", debug=False)
    xt = nc.dram_tensor("xt", [cfg.rows, B], mybir.dt.bfloat16, kind="ExternalInput")
    idx = nc.dram_tensor("idx", [cfg.ntiles, 128, cfg.idx_cols], mybir.dt.int32,
                         kind="ExternalInput")
    idx4 = nc.dram_tensor("idx4", [cfg.nsuper, 128, cfg.idx4_cols], mybir.dt.int32,
                          kind="ExternalInput")
    wt = nc.dram_tensor("wt", [128, 5 * KN], mybir.dt.bfloat16, kind="ExternalInput")
    out = nc.dram_tensor("out", [KN, cfg.px_nc, B], mybir.dt.float32,
                         kind="ExternalOutput")
    NT, NS, PT, NSUP = cfg.ntiles, cfg.nsub, cfg.px_tile, cfg.nsuper
    GPT = 16 * 128          # gsem increments per tile
    G4S = 16 * 64           # g4 increments per super

    from contextlib import ExitStack
    with ExitStack() as _es:
        block = _es.enter_context(nc.Block())
        def _sb(name, shape, dt):
            return _es.enter_context(nc.sbuf_tensor(name, shape, dt))
        def _psum(name, shape, dt):
            return _es.enter_context(nc.psum_tensor(name, shape, dt))
        def _sem(name):
            return _es.enter_context(nc.semaphore(name))
        g0 = _sb("g0", [128, cfg.gw], mybir.dt.bfloat16)
        g1 = _sb("g1", [128, cfg.gw], mybir.dt.bfloat16)
        g4a = _sb("g4a", [64, cfg.gw], mybir.dt.bfloat16)
        g4b = _sb("g4b", [64, cfg.gw], mybir.dt.bfloat16)
        ix0 = _sb("ix0", [128, cfg.idx_cols], mybir.dt.int32)
        ix1 = _sb("ix1", [128, cfg.idx_cols], mybir.dt.int32)
        ix4a = _sb("ix4a", [128, cfg.idx4_cols], mybir.dt.int32)
        ix4b = _sb("ix4b", [128, cfg.idx4_cols], mybir.dt.int32)
        wt_sb = _sb("wt_sb", [128, 5 * KN], mybir.dt.bfloat16)
        stage = _sb("stage", [KN, PT * B], mybir.dt.float32)
        ps0 = _psum("ps0", [KN, 512], mybir.dt.float32)
        ps1 = _psum("ps1", [KN, 512], mybir.dt.float32)
        ps2 = _psum("ps2", [KN, 512], mybir.dt.float32)
        ps3 = _psum("ps3", [KN, 512], mybir.dt.float32)
        ld0, ld1, ld4a, ld4b = _sem("ld0"), _sem("ld1"), _sem("ld4a"), _sem("ld4b")
        wld = _sem("wld")
        ga, gb, g4 = _sem("ga"), _sem("gb"), _sem("g4")
        mm, cp, st = _sem("mm"), _sem("cp"), _sem("st")
        gbuf = [g0, g1]
        g4buf = [g4a, g4b]
        ixbuf = [ix0, ix1]
        ix4buf = [ix4a, ix4b]
        psum = [ps0, ps1, ps2, ps3]
        ldsem = [ld0, ld1]
        ld4sem = [ld4a, ld4b]
        gsem = [ga, gb]

        @block.sync
        def _(sync):
            sync.dma_start(wt_sb[:], wt[:]).then_inc(wld, 16)
            sync.dma_start(ix0[:], idx[0]).then_inc(ld0, 16)
            if NT > 1:
                sync.dma_start(ix1[:], idx[1]).then_inc(ld1, 16)
            sync.dma_start(ix4a[:], idx4[0]).then_inc(ld4a, 16)
            if NSUP > 1:
                sync.dma_start(ix4b[:], idx4[1]).then_inc(ld4b, 16)
            for t in range(NT):
                sync.wait_ge(cp, NS * (t + 1))
                sync.dma_start(
                    out[:, t * PT:(t + 1) * PT, :],
                    stage[:].rearrange("n (p b) -> n p b", p=PT, b=B),
                ).then_inc(st, 16)
                if t + 2 < NT:
                    sync.wait_ge(gsem[t % 2], GPT * (t // 2 + 1))
                    sync.dma_start(ixbuf[t % 2][:], idx[t + 2]).then_inc(
                        ldsem[t % 2], 16)
                if t % 4 == 0 and t // 4 + 2 < NSUP:
                    s = t // 4
                    sync.wait_ge(g4, G4S * (s + 1))
                    sync.dma_start(ix4buf[s % 2][:], idx4[s + 2]).then_inc(
                        ld4sem[s % 2], 16)

        @block.gpsimd
        def _(gpsimd):
            with gpsimd.register("bc") as bc:
                gpsimd.reg_mov(bc, cfg.rows - 1)
                for t in range(NT):
                    gpsimd.wait_ge(ldsem[t % 2], 16 * (t // 2 + 1))
                    if t >= 2:
                        gpsimd.wait_ge(mm, NS * (t - 1))
                    gt = gbuf[t % 2]
                    ixt = ixbuf[t % 2]
                    for rho in range(128):
                        gpsimd.indirect_dma_start(
                            out=gt[rho:rho + 1, :].rearrange(
                                "p (j k) -> p j k", j=LSLOT, k=B),
                            out_offset=None,
                            in_=xt[:],
                            in_offset=bass.IndirectOffsetOnAxis(
                                ap=ixt[:, rho * WCOLS:(rho + 1) * WCOLS],
                                axis=0),
                            bounds_check=bc,
                            oob_is_err=False,
                        ).then_inc(gsem[t % 2], 16)
                    if t % 4 == 0:
                        s = t // 4
                        gpsimd.wait_ge(ld4sem[s % 2], 16 * (s // 2 + 1))
                        ix4t = ix4buf[s % 2]
                        g4t = g4buf[s % 2]
                        for rho in range(64):
                            gpsimd.indirect_dma_start(
                                out=g4t[rho:rho + 1, :].rearrange(
                                    "p (j k) -> p j k", j=LSLOT, k=B),
                                out_offset=None,
                                in_=xt[:],
                                in_offset=bass.IndirectOffsetOnAxis(
                                    ap=ix4t[:, rho * WCOLS:(rho + 1) * WCOLS],
                                    axis=0),
                                bounds_check=bc,
                                oob_is_err=False,
                            ).then_inc(g4, 16)

        @block.tensor
        def _(tensor):
            tensor.wait_ge(wld, 16)
            for t in range(NT):
                for sub in range(NS):
                    gs = t * NS + sub
                    if sub == 0:
                        tensor.wait_ge(gsem[t % 2], GPT * (t // 2 + 1))
                        tensor.wait_ge(g4, G4S * (t // 4 + 1))
                    if gs >= 4:
                        tensor.wait_ge(cp, gs - 3)
                    ps = psum[gs % 4]
                    gt = gbuf[t % 2]
                    g4t = g4buf[(t // 4) % 2]
                    inst = None
                    for c in range(4):
                        base = ((c * NS + sub) * BLK + PAD) * B
                        inst = tensor.matmul(
                            ps[:],
                            wt_sb[0:128, c * KN:(c + 1) * KN],
                            gt[0:128, base:base + GRP * B],
                            start=(c == 0),
                            stop=False,
                        )
                    gidx = (t % 4) * NS + sub
                    base4 = (gidx * BLK + PAD) * B
                    inst = tensor.matmul(
                        ps[:],
                        wt_sb[0:64, 4 * KN:5 * KN],
                        g4t[0:64, base4:base4 + GRP * B],
                        start=False,
                        stop=True,
                    )
                    inst.then_inc(mm, 1)

        @block.scalar
        def _(scalar):
            for t in range(NT):
                for sub in range(NS):
                    gs = t * NS + sub
                    scalar.wait_ge(mm, gs + 1)
                    if sub == 0 and t > 0:
                        scalar.wait_ge(st, 16 * t)
                    scalar.copy(
                        stage[:, sub * 512:(sub + 1) * 512],
                        psum[gs % 4][:],
                    ).then_inc(cp, 1)

    nc.compile()
    return nc


def _wrap(stream, sim_order):
    if sim_order:
        return stream.reshape(128, WCOLS)
    return stream.reshape(WCOLS, 128).T


def host_prep_idx(ht_slice, cfg: Cfg, sim_order=False):
    """ht_slice [px_nc, 576] int32 -> (idx [ntiles,128,640], idx4 [nsuper,128,320])."""
    PT, NS = cfg.px_tile, cfg.nsub
    idx = np.zeros((cfg.ntiles, 128, cfg.idx_cols), dtype=np.int32)
    idx4 = np.zeros((cfg.nsuper, 128, cfg.idx4_cols), dtype=np.int32)
    for t in range(cfg.ntiles):
        S = ht_slice[t * PT:(t + 1) * PT].T          # [576, PT]
        for rho in range(128):
            ent = S[rho:512:128].reshape(4 * NS, GRP)    # chunks 0..3, (c,sub) x 32
            blocks = np.zeros((16, BLK), dtype=np.int32)
            blocks[:, PAD:] = ent
            idx[t, :, rho * WCOLS:(rho + 1) * WCOLS] = _wrap(
                blocks.reshape(-1), sim_order)
    for s in range(cfg.nsuper):
        S4 = ht_slice[s * 4 * PT:(s + 1) * 4 * PT, 512:].T   # [64, 512px]
        for rho in range(64):
            ent = S4[rho].reshape(16, GRP)               # (tile-in-super, sub) x 32
            blocks = np.zeros((16, BLK), dtype=np.int32)
            blocks[:, PAD:] = ent
            idx4[s, :, rho * WCOLS:(rho + 1) * WCOLS] = _wrap(
                blocks.reshape(-1), sim_order)
    return idx, idx4


def host_prep_w(weights):
    import ml_dtypes
    w = np.asarray(weights, dtype=np.float32)
    wt = np.zeros((128, 5 * KN), dtype=ml_dtypes.bfloat16)
    for c in range(4):
        wt[:, c * KN:(c + 1) * KN] = w[:, c * 128:(c + 1) * 128].T
    wt[:64, 4 * KN:5 * KN] = w[:, 512:576].T
    return wt


def kernel(x, hashtable, weights, _trace=False):
    cfg = _cached.setdefault("cfg", Cfg())
    if "nc" not in _cached:
        _cached["nc"] = build(cfg)
    nc = _cached["nc"]
    import ml_dtypes
    xt = np.ascontiguousarray(
        np.asarray(x, dtype=np.float32).reshape(B, C * H * W).T).astype(
            ml_dtypes.bfloat16)
    ht = np.asarray(hashtable).astype(np.int32)
    wt = host_prep_w(weights)
    in_maps = []
    for i in range(NCORES):
        idx_i, idx4_i = host_prep_idx(ht[i * cfg.px_nc:(i + 1) * cfg.px_nc], cfg)
        in_maps.append({"xt": xt, "idx": idx_i, "idx4": idx4_i, "wt": wt})
    res = run_bass_kernel_spmd(nc, in_maps, core_ids=list(range(NCORES)),
                               trace=_trace)
    outs = [res.results[i]["out"] for i in range(NCORES)]
    full = np.concatenate(outs, axis=1)
    out = full.transpose(2, 0, 1).reshape(B, KN, H, W)
    if _trace:
        kernel.last_exec_time_ns = res.exec_time_ns
    return np.ascontiguousarray(out)


# revision 12
# speedup vs baseline: 3.8726x; 3.8726x over previous
"""Trainium2 Bass kernel for hash-indexed im2col gather + GEMM.

Reference computation:
    gathered = x.reshape(B, -1)[:, hashtable]        # [B, HW, K*C]
    out = einsum('nq,bpq->bnp', weights, gathered)   # [B, KN, HW]
    return out.reshape(B, KN, H, W)

Strategy (pixel-parallel over 8 NeuronCores; all 16 batch images processed
together since the hashtable is shared across the batch):
  - Host: transpose x to xt [1048576, 16] f32 so each flat index is one
    contiguous 64B row holding all 16 batch values.
  - Each NC owns 2048 pixels, processed in 128-pixel tiles. For each tile
    and each SBUF partition rho, one SWDGE indirect DMA gathers that
    partition's contraction rows (q = c*128 + rho, c = 0..3) for all tile
    pixels, landing directly in matmul-ready layout G[rho, (c, sub, px, b)].
    The 5th contraction chunk (q = 512+rho4, rho4 < 64) is gathered by
    separate instructions packing 4 tiles each.
  - The SWDGE indirect path corrupts the first descriptor of each DMA
    engine's run (it reads the index L/16*e... positions displaced), i.e.
    every L/16-th stream slot. All instructions are exactly 640 slots so
    the corruption period is 40, and streams are built from 40-slot blocks
    of [8 pad + 32 real]: corrupted slots always land on pads.
  - TensorE contracts q in 5 chunks (4x128 + 64) accumulating in PSUM
    [64, 32px*16b]; ScalarE copies PSUM->SBUF; HWDGE writes DRAM.
"""
import sys
import numpy as np

if "/opt/trn_rl_repo" not in sys.path:
    sys.path.insert(0, "/opt/trn_rl_repo")

import concourse.bacc as bacc
import concourse.bass as bass
import concourse.mybir as mybir
from concourse._compat import get_trn_type
from concourse.bass_utils import run_bass_kernel_spmd

B, C, H, W = 16, 64, 128, 128
HW = H * W
Q = 576
KN = 64
NCORES = 8
BLK = 40          # stream block: 8 pad + 32 real rows
PAD = 8
GRP = 32          # pixels per matmul sub-group
LSLOT = 640       # slots per gather instruction (16 blocks)
WCOLS = LSLOT // 128   # 5 idx columns per instruction


class Cfg:
    def __init__(self, px_nc=2048, rows=C * H * W):
        self.px_nc = px_nc
        self.px_tile = 128
        self.ntiles = px_nc // self.px_tile           # 16
        self.nsuper = self.ntiles // 4                # 4
        self.nsub = self.px_tile // GRP               # 4
        self.idx_cols = 128 * WCOLS                   # 640 (main, per tile)
        self.idx4_cols = 64 * WCOLS                   # 320 (chunk4, per super)
        self.gw = LSLOT * (B // 2)                    # packed bf16-pair f32 words
        self.rows = rows


_cached = {}


def build(cfg: Cfg):
    nc = bacc.Bacc(get_trn_type() or "TRN2", debug=False)
    xt = nc.dram_tensor("xt", [cfg.rows, B // 2], mybir.dt.float32, kind="ExternalInput")
    idx = nc.dram_tensor("idx", [cfg.ntiles, 128, cfg.idx_cols], mybir.dt.int32,
                         kind="ExternalInput")
    idx4 = nc.dram_tensor("idx4", [cfg.nsuper, 128, cfg.idx4_cols], mybir.dt.int32,
                          kind="ExternalInput")
    wt = nc.dram_tensor("wt", [128, 5 * KN], mybir.dt.bfloat16, kind="ExternalInput")
    out = nc.dram_tensor("out", [KN, cfg.px_nc, B], mybir.dt.float32,
                         kind="ExternalOutput")
    NT, NS, PT, NSUP = cfg.ntiles, cfg.nsub, cfg.px_tile, cfg.nsuper
    GPT = 16 * 128          # gsem increments per tile
    G4S = 16 * 64           # g4 increments per super

    from contextlib import ExitStack
    with ExitStack() as _es:
        block = _es.enter_context(nc.Block())
        def _sb(name, shape, dt):
            return _es.enter_context(nc.sbuf_tensor(name, shape, dt))
        def _psum(name, shape, dt):
            return _es.enter_context(nc.psum_tensor(name, shape, dt))
        def _sem(name):
            return _es.enter_context(nc.semaphore(name))
        g0 = _sb("g0", [128, cfg.gw], mybir.dt.float32)
        g1 = _sb("g1", [128, cfg.gw], mybir.dt.float32)
        g4a = _sb("g4a", [64, cfg.gw], mybir.dt.float32)
        g4b = _sb("g4b", [64, cfg.gw], mybir.dt.float32)
        ix0 = _sb("ix0", [128, cfg.idx_cols], mybir.dt.int32)
        ix1 = _sb("ix1", [128, cfg.idx_cols], mybir.dt.int32)
        ix4a = _sb("ix4a", [128, cfg.idx4_cols], mybir.dt.int32)
        ix4b = _sb("ix4b", [128, cfg.idx4_cols], mybir.dt.int32)
        wt_sb = _sb("wt_sb", [128, 5 * KN], mybir.dt.bfloat16)
        stage = _sb("stage", [KN, PT * B], mybir.dt.float32)
        ps0 = _psum("ps0", [KN, 512], mybir.dt.float32)
        ps1 = _psum("ps1", [KN, 512], mybir.dt.float32)
        ps2 = _psum("ps2", [KN, 512], mybir.dt.float32)
        ps3 = _psum("ps3", [KN, 512], mybir.dt.float32)
        ld0, ld1, ld4a, ld4b = _sem("ld0"), _sem("ld1"), _sem("ld4a"), _sem("ld4b")
        wld = _sem("wld")
        ga, gb, g4 = _sem("ga"), _sem("gb"), _sem("g4")
        mm, cp, st = _sem("mm"), _sem("cp"), _sem("st")
        gbuf = [g0, g1]
        g4buf = [g4a, g4b]
        ixbuf = [ix0, ix1]
        ix4buf = [ix4a, ix4b]
        psum = [ps0, ps1, ps2, ps3]
        ldsem = [ld0, ld1]
        ld4sem = [ld4a, ld4b]
        gsem = [ga, gb]

        @block.sync
        def _(sync):
            sync.dma_start(wt_sb[:], wt[:]).then_inc(wld, 16)
            sync.dma_start(ix0[:], idx[0]).then_inc(ld0, 16)
            if NT > 1:
                sync.dma_start(ix1[:], idx[1]).then_inc(ld1, 16)
            sync.dma_start(ix4a[:], idx4[0]).then_inc(ld4a, 16)
            if NSUP > 1:
                sync.dma_start(ix4b[:], idx4[1]).then_inc(ld4b, 16)
            for t in range(NT):
                sync.wait_ge(cp, NS * (t + 1))
                sync.dma_start(
                    out[:, t * PT:(t + 1) * PT, :],
                    stage[:].rearrange("n (p b) -> n p b", p=PT, b=B),
                ).then_inc(st, 16)
                if t + 2 < NT:
                    sync.wait_ge(gsem[t % 2], GPT * (t // 2 + 1))
                    sync.dma_start(ixbuf[t % 2][:], idx[t + 2]).then_inc(
                        ldsem[t % 2], 16)
                if t % 4 == 0 and t // 4 + 2 < NSUP:
                    s = t // 4
                    sync.wait_ge(g4, G4S * (s + 1))
                    sync.dma_start(ix4buf[s % 2][:], idx4[s + 2]).then_inc(
                        ld4sem[s % 2], 16)

        @block.gpsimd
        def _(gpsimd):
            with gpsimd.register("bc") as bc:
                gpsimd.reg_mov(bc, cfg.rows - 1)
                for t in range(NT):
                    gpsimd.wait_ge(ldsem[t % 2], 16 * (t // 2 + 1))
                    if t >= 2:
                        gpsimd.wait_ge(mm, NS * (t - 1))
                    gt = gbuf[t % 2]
                    ixt = ixbuf[t % 2]
                    for rho in range(128):
                        gpsimd.indirect_dma_start(
                            out=gt[rho:rho + 1, :].rearrange(
                                "p (j k) -> p j k", j=LSLOT, k=B // 2),
                            out_offset=None,
                            in_=xt[:],
                            in_offset=bass.IndirectOffsetOnAxis(
                                ap=ixt[:, rho * WCOLS:(rho + 1) * WCOLS],
                                axis=0),
                            bounds_check=bc,
                            oob_is_err=False,
                        ).then_inc(gsem[t % 2], 16)
                    if t % 4 == 0:
                        s = t // 4
                        gpsimd.wait_ge(ld4sem[s % 2], 16 * (s // 2 + 1))
                        ix4t = ix4buf[s % 2]
                        g4t = g4buf[s % 2]
                        for rho in range(64):
                            gpsimd.indirect_dma_start(
                                out=g4t[rho:rho + 1, :].rearrange(
                                    "p (j k) -> p j k", j=LSLOT, k=B // 2),
                                out_offset=None,
                                in_=xt[:],
                                in_offset=bass.IndirectOffsetOnAxis(
                                    ap=ix4t[:, rho * WCOLS:(rho + 1) * WCOLS],
                                    axis=0),
                                bounds_check=bc,
                                oob_is_err=False,
                            ).then_inc(g4, 16)

        @block.tensor
        def _(tensor):
            tensor.wait_ge(wld, 16)
            for t in range(NT):
                for sub in range(NS):
                    gs = t * NS + sub
                    if sub == 0:
                        tensor.wait_ge(gsem[t % 2], GPT * (t // 2 + 1))
                        tensor.wait_ge(g4, G4S * (t // 4 + 1))
                    if gs >= 4:
                        tensor.wait_ge(cp, gs - 3)
                    ps = psum[gs % 4]
                    gt = gbuf[t % 2]
                    g4t = g4buf[(t // 4) % 2]
                    inst = None
                    for c in range(4):
                        base = ((c * NS + sub) * BLK + PAD) * (B // 2)
                        inst = tensor.matmul(
                            ps[:],
                            wt_sb[0:128, c * KN:(c + 1) * KN],
                            gt[0:128, base:base + GRP * (B // 2)].bitcast(
                                mybir.dt.bfloat16),
                            start=(c == 0),
                            stop=False,
                        )
                    gidx = (t % 4) * NS + sub
                    base4 = (gidx * BLK + PAD) * (B // 2)
                    inst = tensor.matmul(
                        ps[:],
                        wt_sb[0:64, 4 * KN:5 * KN],
                        g4t[0:64, base4:base4 + GRP * (B // 2)].bitcast(
                            mybir.dt.bfloat16),
                        start=False,
                        stop=True,
                    )
                    inst.then_inc(mm, 1)

        @block.scalar
        def _(scalar):
            for t in range(NT):
                for sub in range(NS):
                    gs = t * NS + sub
                    scalar.wait_ge(mm, gs + 1)
                    if sub == 0 and t > 0:
                        scalar.wait_ge(st, 16 * t)
                    scalar.copy(
                        stage[:, sub * 512:(sub + 1) * 512],
                        psum[gs % 4][:],
                    ).then_inc(cp, 1)

    nc.compile()
    return nc


def _wrap(stream, sim_order):
    if sim_order:
        return stream.reshape(128, WCOLS)
    return stream.reshape(WCOLS, 128).T


def host_prep_idx(ht_slice, cfg: Cfg, sim_order=False):
    """ht_slice [px_nc, 576] int32 -> (idx [ntiles,128,640], idx4 [nsuper,128,320])."""
    PT, NS = cfg.px_tile, cfg.nsub
    idx = np.zeros((cfg.ntiles, 128, cfg.idx_cols), dtype=np.int32)
    idx4 = np.zeros((cfg.nsuper, 128, cfg.idx4_cols), dtype=np.int32)
    for t in range(cfg.ntiles):
        S = ht_slice[t * PT:(t + 1) * PT].T          # [576, PT]
        for rho in range(128):
            ent = S[rho:512:128].reshape(4 * NS, GRP)    # chunks 0..3, (c,sub) x 32
            blocks = np.zeros((16, BLK), dtype=np.int32)
            blocks[:, PAD:] = ent
            idx[t, :, rho * WCOLS:(rho + 1) * WCOLS] = _wrap(
                blocks.reshape(-1), sim_order)
    for s in range(cfg.nsuper):
        S4 = ht_slice[s * 4 * PT:(s + 1) * 4 * PT, 512:].T   # [64, 512px]
        for rho in range(64):
            ent = S4[rho].reshape(16, GRP)               # (tile-in-super, sub) x 32
            blocks = np.zeros((16, BLK), dtype=np.int32)
            blocks[:, PAD:] = ent
            idx4[s, :, rho * WCOLS:(rho + 1) * WCOLS] = _wrap(
                blocks.reshape(-1), sim_order)
    return idx, idx4


def host_prep_w(weights):
    import ml_dtypes
    w = np.asarray(weights, dtype=np.float32)
    wt = np.zeros((128, 5 * KN), dtype=ml_dtypes.bfloat16)
    for c in range(4):
        wt[:, c * KN:(c + 1) * KN] = w[:, c * 128:(c + 1) * 128].T
    wt[:64, 4 * KN:5 * KN] = w[:, 512:576].T
    return wt


def kernel(x, hashtable, weights, _trace=False):
    cfg = _cached.setdefault("cfg", Cfg())
    if "nc" not in _cached:
        _cached["nc"] = build(cfg)
    nc = _cached["nc"]
    import ml_dtypes
    xt = np.ascontiguousarray(
        np.asarray(x, dtype=np.float32).reshape(B, C * H * W).T).astype(
            ml_dtypes.bfloat16).view(np.float32)
    ht = np.asarray(hashtable).astype(np.int32)
    wt = host_prep_w(weights)
    in_maps = []
    for i in range(NCORES):
        idx_i, idx4_i = host_prep_idx(ht[i * cfg.px_nc:(i + 1) * cfg.px_nc], cfg)
        in_maps.append({"xt": xt, "idx": idx_i, "idx4": idx4_i, "wt": wt})
    res = run_bass_kernel_spmd(nc, in_maps, core_ids=list(range(NCORES)),
                               trace=_trace)
    outs = [res.results[i]["out"] for i in range(NCORES)]
    full = np.concatenate(outs, axis=1)
    out = full.transpose(2, 0, 1).reshape(B, KN, H, W)
    if _trace:
        kernel.last_exec_time_ns = res.exec_time_ns
    return np.ascontiguousarray(out)
